# revision 18
# baseline (speedup 1.0000x reference)
"""Trainium2 Bass kernel for LogWeightedDICELossMultiClass3D.

Input: output (4,3,64,192,192) f32, masks (same), loss_threshold scalar.
Strategy: shard H=192 into 8 slabs of 24 rows (one per NeuronCore, with a
1-row halo clamped on host). Each core reduces its slab of all 12 (b,c)
volumes to partial sums:
  sum(m), sum(o), sum(o*m), sum((o>thr)==m), sum(sobel_edge)
The host combines the tiny partials into the loss.

Device layout per core: 6 supertiles of 128 partitions = 2 volumes x 64 z.
Free dim = 26 H-rows (24 + 2 halo) x 192 W, flat (4992 elements).

v3 engine split (all maps bf16 via SWDGE cast-DMA, f32 never hits SBUF):
  DVE : ts=(o>thr) [4x], W-derivative d [2x], eq=(ts==m) [2x], om=o*m [2x]
  ACT : sum(m) copy-accum, edge sigmoids with accum
  PE  : sobel Z(H)-smoothing matmuls + per-volume sums of o/eq/om as bf16
        ones-matmuls accumulated across ALL supertiles into persistent
        [12,512] PSUM banks (per-supertile volume-selector stationaries)
grad is integer-valued, so edge=(grad>0) is computed exactly by a saturated
Sigmoid(100*grad-50) on the ACT engine with a fused per-partition reduction.
"""

import numpy as np
import ml_dtypes

import concourse.bacc as bacc
import concourse.bass as bass
import concourse.tile as tile
from concourse import mybir
from concourse.bass_utils import run_bass_kernel_spmd

F32 = mybir.dt.float32
BF16 = mybir.dt.bfloat16
ALU = mybir.AluOpType
ACTF = mybir.ActivationFunctionType

B, C, Z, H, W = 4, 3, 64, 192, 192
NV = B * C            # 12 volumes
NCORES = 8
HC = H // NCORES      # 24 H-rows per core
NS = NV // 2          # 6 supertiles (2 volumes each)
FH = HC + 2           # 26 rows incl halo
FW = FH * W           # 4992 free elements per partition (o / ts / d)
CW = HC * W           # 4608 center free elements (m / eq / om)
C0 = W                # flat offset of center region (row 1)
VOX = Z * H * W
# grad chunking: [128,1024] psum tiles (2 banks), last chunk 512
GCH = [1024, 1024, 1024, 1024, 512]
NCH = len(GCH)

_CACHE = {}


def _band64():
    """[1,2,1] smoothing matrix with scipy 'reflect' (np symmetric) ends."""
    M = np.zeros((Z, Z), dtype=np.float64)
    for i in range(Z):
        M[i, i] = 2.0
        if i > 0:
            M[i, i - 1] += 1.0
        else:
            M[i, i] += 1.0
        if i < Z - 1:
            M[i, i + 1] += 1.0
        else:
            M[i, i] += 1.0
    return M


def _consts():
    Bz = _band64()
    blk = np.zeros((128, 128), dtype=np.float64)
    blk[:64, :64] = Bz
    blk[64:, 64:] = Bz
    bz1 = blk.astype(ml_dtypes.bfloat16)          # weights 1,2,3 - exact
    bz2 = (2.0 * blk).astype(ml_dtypes.bfloat16)  # weights 2,4,6 - exact
    # per-supertile volume selectors: vsel[p, s*NV+v] = 1 iff partition p of
    # supertile s belongs to volume v (vol 2s: p<64, vol 2s+1: p>=64)
    vsel = np.zeros((128, NS, NV), dtype=ml_dtypes.bfloat16)
    for s in range(NS):
        vsel[:64, s, 2 * s] = 1.0
        vsel[64:, s, 2 * s + 1] = 1.0
    return bz1, bz2, vsel.reshape(128, NS * NV)


def _build_program():
    nc = bacc.Bacc("TRN2", target_bir_lowering=False, debug=False,
                   num_devices=NCORES)
    o_d = nc.dram_tensor("o", [NV * Z, FW], F32, kind="ExternalInput").ap()
    m_d = nc.dram_tensor("m", [NV * Z, CW], F32, kind="ExternalInput").ap()
    thr_d = nc.dram_tensor("thr", [128, 1], F32, kind="ExternalInput").ap()
    bz1_d = nc.dram_tensor("bz1", [128, 128], BF16, kind="ExternalInput").ap()
    bz2_d = nc.dram_tensor("bz2", [128, 128], BF16, kind="ExternalInput").ap()
    vsel_d = nc.dram_tensor("vsel", [128, NS * NV], BF16,
                            kind="ExternalInput").ap()
    part_d = nc.dram_tensor("partials", [128, 6 + 5 * NS], F32,
                            kind="ExternalOutput").ap()
    vs_d = nc.dram_tensor("vsums", [NV, 3 * 512], F32,
                          kind="ExternalOutput").ap()

    from contextlib import ExitStack
    with tile.TileContext(nc) as tc, ExitStack() as ctx:
        consts = ctx.enter_context(tc.tile_pool(name="consts", bufs=1))
        io = ctx.enter_context(tc.tile_pool(name="io", bufs=2))
        mid = ctx.enter_context(tc.tile_pool(name="mid", bufs=2))
        slots = ctx.enter_context(tc.tile_pool(name="slots", bufs=1))
        scr = ctx.enter_context(tc.tile_pool(name="scr", bufs=2))
        gps = ctx.enter_context(tc.tile_pool(name="gps", bufs=2, space="PSUM"))
        acc = ctx.enter_context(tc.tile_pool(name="acc", bufs=1, space="PSUM"))

        thr_t = consts.tile([128, 1], F32)
        nc.default_dma_engine.dma_start(out=thr_t, in_=thr_d)
        bz1_t = consts.tile([128, 128], BF16)
        nc.default_dma_engine.dma_start(out=bz1_t, in_=bz1_d)
        bz2_t = consts.tile([128, 128], BF16)
        nc.default_dma_engine.dma_start(out=bz2_t, in_=bz2_d)
        vsel_t = consts.tile([128, NS * NV], BF16)
        nc.default_dma_engine.dma_start(out=vsel_t, in_=vsel_d)
        nbias_t = consts.tile([128, 1], F32)
        nc.vector.memset(nbias_t, -50.0)

        msum = slots.tile([128, NS], F32)
        edgesum = slots.tile([128, 4 * NS + NS], F32)  # 5 chunks per st
        vs_sb = slots.tile([NV, 3 * 512], F32)

        # persistent per-volume accumulators (1 psum bank each)
        osum_p = acc.tile([NV, 512], F32, name="osum_p")
        eqsum_p = acc.tile([NV, 512], F32, name="eqsum_p")
        omsum_p = acc.tile([NV, 512], F32, name="omsum_p")

        for s in range(NS):
            vsel_s = vsel_t[:, NV * s:NV * (s + 1)]
            # cast-DMA: DRAM f32 -> SBUF bf16 (SWDGE does the conversion)
            o_t = io.tile([128, FW], BF16, tag="o", name=f"o{s}")
            nc.gpsimd.dma_start(
                out=o_t, in_=o_d[128 * s:128 * (s + 1), :])
            m_t = io.tile([128, CW], BF16, tag="m", name=f"m{s}")
            nc.gpsimd.dma_start(
                out=m_t, in_=m_d[128 * s:128 * (s + 1), :])

            fl = (s == 0)
            ll = (s == NS - 1)

            # per-volume sum(o): ones-matmuls, only need o_t -> PE starts
            # as soon as the o DMA lands
            for k in range(9):
                nc.tensor.matmul(
                    out=osum_p, lhsT=vsel_s,
                    rhs=o_t[:, C0 + 512 * k:C0 + 512 * (k + 1)],
                    start=(fl and k == 0), stop=(ll and k == 8))

            # ts = (o > thr), bf16 0/1 (4x DVE mode)
            ts_t = mid.tile([128, FW], BF16, tag="ts", name=f"ts{s}")
            nc.vector.tensor_scalar(
                out=ts_t, in0=o_t, scalar1=thr_t, scalar2=None,
                op0=ALU.is_gt)

            # d = W-derivative of ts (symmetric boundary), cols permuted:
            # cols 0..189 = d[w=1..190], col 190 = d[w=0], col 191 = d[w=191]
            # computed BEFORE eq/om so the grad matmuls never wait on the
            # m DMA
            d_t = mid.tile([128, FW], BF16, tag="d", name=f"d{s}")
            ts3 = ts_t.rearrange("p (a b) -> p a b", b=W)
            d3 = d_t.rearrange("p (a b) -> p a b", b=W)
            nc.vector.tensor_tensor(
                out=d3[:, :, 0:190], in0=ts3[:, :, 2:192],
                in1=ts3[:, :, 0:190], op=ALU.subtract)
            nc.vector.tensor_tensor(
                out=d3[:, :, 190:192], in0=ts3[:, :, 1::190],
                in1=ts3[:, :, 0::190], op=ALU.subtract)

            # grad = S_Z(S_H(d)) via 3 H-shifted banded matmuls into PSUM
            base = C0
            for j, gw in enumerate(GCH):
                g_t = gps.tile([128, 1024], F32, tag="g", name=f"g{s}_{j}")
                for di, (lhs, doff) in enumerate(
                        [(bz1_t, -W), (bz1_t, W), (bz2_t, 0)]):
                    for c0 in range(0, gw, 512):
                        off = base + doff + c0
                        nc.tensor.matmul(
                            out=g_t[:, c0:c0 + 512], lhsT=lhs,
                            rhs=d_t[:, off:off + 512],
                            start=(di == 0), stop=(di == 2))
                # edge = (grad > 0): integer grad, sigmoid saturates
                e_t = scr.tile([128, 1024], BF16, tag="edge",
                               name=f"e{s}_{j}")
                nc.scalar.activation(
                    out=e_t[:, 0:gw], in_=g_t[:, 0:gw], func=ACTF.Sigmoid,
                    scale=100.0, bias=nbias_t,
                    accum_out=edgesum[:, 5 * s + j:5 * s + j + 1])
                base += gw

            # sum(m) on ACT (copy with fused accumulator)
            ma_t = scr.tile([128, CW], BF16, tag="ma", name=f"ma{s}")
            nc.scalar.activation(
                out=ma_t, in_=m_t, func=ACTF.Copy,
                accum_out=msum[:, s:s + 1])

            # eq = (ts == m) then its per-volume sums
            eq_t = scr.tile([128, CW], BF16, tag="eq", name=f"eq{s}")
            nc.vector.tensor_tensor(
                out=eq_t, in0=ts_t[:, C0:C0 + CW], in1=m_t, op=ALU.is_equal)
            for k in range(9):
                nc.tensor.matmul(
                    out=eqsum_p, lhsT=vsel_s,
                    rhs=eq_t[:, 512 * k:512 * (k + 1)],
                    start=(fl and k == 0), stop=(ll and k == 8))

            # om = o * m then its per-volume sums
            om_t = scr.tile([128, CW], BF16, tag="om", name=f"om{s}")
            nc.vector.tensor_tensor(
                out=om_t, in0=o_t[:, C0:C0 + CW], in1=m_t, op=ALU.mult)
            for k in range(9):
                nc.tensor.matmul(
                    out=omsum_p, lhsT=vsel_s,
                    rhs=om_t[:, 512 * k:512 * (k + 1)],
                    start=(fl and k == 0), stop=(ll and k == 8))

        # drain persistent accumulators to SBUF, then DRAM
        nc.vector.tensor_copy(vs_sb[:, 0:512], osum_p)
        nc.vector.tensor_copy(vs_sb[:, 512:1024], eqsum_p)
        nc.vector.tensor_copy(vs_sb[:, 1024:1536], omsum_p)
        nc.default_dma_engine.dma_start(out=vs_d, in_=vs_sb)
        nc.default_dma_engine.dma_start(out=part_d[:, 0:6], in_=msum)
        nc.default_dma_engine.dma_start(out=part_d[:, 6:36], in_=edgesum)

    nc.compile()
    return nc


def _get_program():
    if "nc" not in _CACHE:
        _CACHE["nc"] = _build_program()
    return _CACHE["nc"]


def _make_in_maps(output, masks, loss_threshold):
    o5 = np.ascontiguousarray(np.asarray(output, dtype=np.float32)).reshape(
        NV, Z, H, W)
    m5 = np.ascontiguousarray(np.asarray(masks, dtype=np.float32)).reshape(
        NV, Z, H, W)
    thr = np.full((128, 1), np.float32(np.asarray(loss_threshold)), np.float32)
    bz1, bz2, vsel = _consts()
    in_maps = []
    for c in range(NCORES):
        h0 = HC * c
        idx = np.clip(np.arange(h0 - 1, h0 + HC + 1), 0, H - 1)
        o_sh = np.ascontiguousarray(o5[:, :, idx, :]).reshape(NV * Z, FW)
        m_sh = np.ascontiguousarray(m5[:, :, h0:h0 + HC, :]).reshape(NV * Z, CW)
        in_maps.append({
            "o": o_sh, "m": m_sh, "thr": thr,
            "bz1": bz1, "bz2": bz2, "vsel": vsel,
        })
    return in_maps


def _combine(results):
    """Host-side tiny reduction: per-core partials -> loss scalar."""
    sum_m = np.zeros(NV)
    sum_eq = np.zeros(NV)
    sum_om = np.zeros(NV)
    sum_o = np.zeros(NV)
    sum_edge = np.zeros(NV)
    for r in results:
        p = np.asarray(r["partials"], dtype=np.float64)
        vs = np.asarray(r["vsums"], dtype=np.float64).reshape(NV, 3, 512)
        # [p, s]: volume = 2s + p//64, z = p%64
        sum_m += p[:, 0:6].reshape(2, 64, NS).sum(1).T.reshape(-1)
        sum_edge += (p[:, 6:36].reshape(2, 64, NS, 5).sum(axis=(1, 3))
                     .T.reshape(-1))
        sum_o += vs[:, 0].sum(-1)
        sum_eq += vs[:, 1].sum(-1)
        sum_om += vs[:, 2].sum(-1)

    freq = (sum_m / VOX).reshape(B, C)
    med = np.median(freq, axis=1, keepdims=True)
    w0 = 2.0 * med / (freq.min(axis=1, keepdims=True) + 1e-5)
    cw = (med / (freq + 1e-5)) * sum_eq.reshape(B, C) \
        + w0 * sum_edge.reshape(B, C)
    ps1 = sum_om.reshape(B, C)
    ps2 = (sum_o + sum_m).reshape(B, C)
    nom = (cw * ps1).sum(1)
    denom = (cw * ps2 + 1e-7).sum(1)
    loss = (1.0 - 2.0 * nom / denom).sum() / B
    return np.array([loss], dtype=np.float32)


def run(output, masks, loss_threshold, trace=False, **trace_kwargs):
    nc = _get_program()
    in_maps = _make_in_maps(output, masks, loss_threshold)
    res = run_bass_kernel_spmd(nc, in_maps, list(range(NCORES)),
                               trace=trace, **trace_kwargs)
    return _combine(res.results), res


def kernel(output, masks, loss_threshold):
    loss, _ = run(output, masks, loss_threshold)
    return loss


# revision 22
# speedup vs baseline: 1.2293x; 1.2293x over previous
"""Trainium2 Bass kernel for LogWeightedDICELossMultiClass3D.

Input: output (4,3,64,192,192) f32, masks (same), loss_threshold scalar.
Strategy: shard H=192 into 8 slabs of 24 rows (one per NeuronCore, with a
1-row halo clamped on host). Each core reduces its slab of all 12 (b,c)
volumes to partial sums:
  sum(m), sum(o), sum(o*m), sum((o>thr)==m), sum(sobel_edge)
The host combines the tiny partials into the loss.

Device layout per core: 6 supertiles of 128 partitions = 2 volumes x 64 z.
Free dim = 26 H-rows (24 + 2 halo) x 192 W, flat (4992 elements).

v3 engine split (all maps bf16 via SWDGE cast-DMA, f32 never hits SBUF):
  DVE : ts=(o>thr) [4x], W-derivative d [2x], eq=(ts==m) [2x], om=o*m [2x]
  ACT : sum(m) copy-accum, edge sigmoids with accum
  PE  : sobel Z(H)-smoothing matmuls + per-volume sums of o/eq/om as bf16
        ones-matmuls accumulated across ALL supertiles into persistent
        [12,512] PSUM banks (per-supertile volume-selector stationaries)
grad is integer-valued, so edge=(grad>0) is computed exactly by a saturated
Sigmoid(100*grad-50) on the ACT engine with a fused per-partition reduction.
"""

import numpy as np
import ml_dtypes

import concourse.bacc as bacc
import concourse.bass as bass
import concourse.tile as tile
from concourse import mybir
from concourse.bass_utils import run_bass_kernel_spmd

F32 = mybir.dt.float32
BF16 = mybir.dt.bfloat16
ALU = mybir.AluOpType
ACTF = mybir.ActivationFunctionType

B, C, Z, H, W = 4, 3, 64, 192, 192
NV = B * C            # 12 volumes
NCORES = 8
HC = H // NCORES      # 24 H-rows per core
NS = NV // 2          # 6 supertiles (2 volumes each)
FH = HC + 2           # 26 rows incl halo
FW = FH * W           # 4992 free elements per partition (o / ts / d)
CW = HC * W           # 4608 center free elements (m / eq / om)
C0 = W                # flat offset of center region (row 1)
VOX = Z * H * W
# grad chunking: [128,1024] psum tiles (2 banks), last chunk 512
GCH = [1024, 1024, 1024, 1024, 512]
NCH = len(GCH)

_CACHE = {}


def _band64():
    """[1,2,1] smoothing matrix with scipy 'reflect' (np symmetric) ends."""
    M = np.zeros((Z, Z), dtype=np.float64)
    for i in range(Z):
        M[i, i] = 2.0
        if i > 0:
            M[i, i - 1] += 1.0
        else:
            M[i, i] += 1.0
        if i < Z - 1:
            M[i, i + 1] += 1.0
        else:
            M[i, i] += 1.0
    return M


def _consts():
    Bz = _band64()
    blk = np.zeros((128, 128), dtype=np.float64)
    blk[:64, :64] = Bz
    blk[64:, 64:] = Bz
    bz1 = blk.astype(ml_dtypes.bfloat16)          # weights 1,2,3 - exact
    bz2 = (2.0 * blk).astype(ml_dtypes.bfloat16)  # weights 2,4,6 - exact
    # per-supertile volume selectors: vsel[p, s*NV+v] = 1 iff partition p of
    # supertile s belongs to volume v (vol 2s: p<64, vol 2s+1: p>=64)
    vsel = np.zeros((128, NS, NV), dtype=ml_dtypes.bfloat16)
    for s in range(NS):
        vsel[:64, s, 2 * s] = 1.0
        vsel[64:, s, 2 * s + 1] = 1.0
    # one packed bf16 const block: [bz1 | bz2 | vsel]
    cb = np.concatenate([bz1, bz2, vsel.reshape(128, NS * NV)], axis=1)
    return np.ascontiguousarray(cb)


def _build_program():
    nc = bacc.Bacc("TRN2", target_bir_lowering=False, debug=False,
                   num_devices=NCORES)
    o_d = nc.dram_tensor("o", [NV * Z, FW], F32, kind="ExternalInput").ap()
    m_d = nc.dram_tensor("m", [NV * Z, CW], F32, kind="ExternalInput").ap()
    thr_d = nc.dram_tensor("thr", [128, 1], F32, kind="ExternalInput").ap()
    cb_d = nc.dram_tensor("cb", [128, 256 + NS * NV], BF16,
                          kind="ExternalInput").ap()
    part_d = nc.dram_tensor("partials", [128, 6 + 5 * NS], F32,
                            kind="ExternalOutput").ap()
    vs_d = nc.dram_tensor("vsums", [NV, 3 * 512], F32,
                          kind="ExternalOutput").ap()

    from contextlib import ExitStack
    with tile.TileContext(nc) as tc, ExitStack() as ctx:
        consts = ctx.enter_context(tc.tile_pool(name="consts", bufs=1))
        io = ctx.enter_context(tc.tile_pool(name="io", bufs=2))
        mid = ctx.enter_context(tc.tile_pool(name="mid", bufs=2))
        slots = ctx.enter_context(tc.tile_pool(name="slots", bufs=1))
        scr = ctx.enter_context(tc.tile_pool(name="scr", bufs=2))
        gps = ctx.enter_context(tc.tile_pool(name="gps", bufs=2, space="PSUM"))
        acc = ctx.enter_context(tc.tile_pool(name="acc", bufs=1, space="PSUM"))

        # consts go FIRST on the SWDGE queue (FIFO) so they land before the
        # big input cast-DMAs flood the SDMA engines
        cb_t = consts.tile([128, 256 + NS * NV], BF16)
        nc.gpsimd.dma_start(out=cb_t, in_=cb_d)
        thr_t = consts.tile([128, 1], F32)
        nc.gpsimd.dma_start(out=thr_t, in_=thr_d)
        bz1_t = cb_t[:, 0:128]
        bz2_t = cb_t[:, 128:256]
        vsel_t = cb_t[:, 256:256 + NS * NV]
        nbias_t = consts.tile([128, 1], F32)
        nc.vector.memset(nbias_t, -50.0)

        msum = slots.tile([128, NS], F32)
        edgesum = slots.tile([128, 4 * NS + NS], F32)  # 5 chunks per st
        vs_sb = slots.tile([NV, 3 * 512], F32)

        # persistent per-volume accumulators (1 psum bank each)
        osum_p = acc.tile([NV, 512], F32, name="osum_p")
        eqsum_p = acc.tile([NV, 512], F32, name="eqsum_p")
        omsum_p = acc.tile([NV, 512], F32, name="omsum_p")

        for s in range(NS):
            vsel_s = vsel_t[:, NV * s:NV * (s + 1)]
            # cast-DMA: DRAM f32 -> SBUF bf16 (SWDGE does the conversion)
            o_t = io.tile([128, FW], BF16, tag="o", name=f"o{s}")
            nc.gpsimd.dma_start(
                out=o_t, in_=o_d[128 * s:128 * (s + 1), :])
            m_t = io.tile([128, CW], BF16, tag="m", name=f"m{s}")
            nc.gpsimd.dma_start(
                out=m_t, in_=m_d[128 * s:128 * (s + 1), :])

            fl = (s == 0)
            ll = (s == NS - 1)

            # per-volume sum(o): ones-matmuls, only need o_t -> PE starts
            # as soon as the o DMA lands
            for k in range(9):
                nc.tensor.matmul(
                    out=osum_p, lhsT=vsel_s,
                    rhs=o_t[:, C0 + 512 * k:C0 + 512 * (k + 1)],
                    start=(fl and k == 0), stop=(ll and k == 8))

            # ts = (o > thr), bf16 0/1 (4x DVE mode)
            ts_t = mid.tile([128, FW], BF16, tag="ts", name=f"ts{s}")
            nc.vector.tensor_scalar(
                out=ts_t, in0=o_t, scalar1=thr_t, scalar2=None,
                op0=ALU.is_gt)

            # d = W-derivative of ts (symmetric boundary), cols permuted:
            # cols 0..189 = d[w=1..190], col 190 = d[w=0], col 191 = d[w=191]
            # computed BEFORE eq/om so the grad matmuls never wait on the
            # m DMA
            d_t = mid.tile([128, FW], BF16, tag="d", name=f"d{s}")
            ts3 = ts_t.rearrange("p (a b) -> p a b", b=W)
            d3 = d_t.rearrange("p (a b) -> p a b", b=W)
            nc.vector.tensor_tensor(
                out=d3[:, :, 0:190], in0=ts3[:, :, 2:192],
                in1=ts3[:, :, 0:190], op=ALU.subtract)
            nc.vector.tensor_tensor(
                out=d3[:, :, 190:192], in0=ts3[:, :, 1::190],
                in1=ts3[:, :, 0::190], op=ALU.subtract)

            # grad = S_Z(S_H(d)) via 3 H-shifted banded matmuls into PSUM
            base = C0
            for j, gw in enumerate(GCH):
                g_t = gps.tile([128, 1024], F32, tag="g", name=f"g{s}_{j}")
                for di, (lhs, doff) in enumerate(
                        [(bz1_t, -W), (bz1_t, W), (bz2_t, 0)]):
                    for c0 in range(0, gw, 512):
                        off = base + doff + c0
                        nc.tensor.matmul(
                            out=g_t[:, c0:c0 + 512], lhsT=lhs,
                            rhs=d_t[:, off:off + 512],
                            start=(di == 0), stop=(di == 2))
                # edge = (grad > 0): integer grad, sigmoid saturates
                e_t = scr.tile([128, 1024], BF16, tag="edge",
                               name=f"e{s}_{j}")
                nc.scalar.activation(
                    out=e_t[:, 0:gw], in_=g_t[:, 0:gw], func=ACTF.Sigmoid,
                    scale=100.0, bias=nbias_t,
                    accum_out=edgesum[:, 5 * s + j:5 * s + j + 1])
                base += gw

            # sum(m) on ACT (copy with fused accumulator)
            ma_t = scr.tile([128, CW], BF16, tag="ma", name=f"ma{s}")
            nc.scalar.activation(
                out=ma_t, in_=m_t, func=ACTF.Copy,
                accum_out=msum[:, s:s + 1])

            # eq = (ts == m) then its per-volume sums
            eq_t = scr.tile([128, CW], BF16, tag="eq", name=f"eq{s}")
            nc.vector.tensor_tensor(
                out=eq_t, in0=ts_t[:, C0:C0 + CW], in1=m_t, op=ALU.is_equal)
            for k in range(9):
                nc.tensor.matmul(
                    out=eqsum_p, lhsT=vsel_s,
                    rhs=eq_t[:, 512 * k:512 * (k + 1)],
                    start=(fl and k == 0), stop=(ll and k == 8))

            # om = o * m then its per-volume sums
            om_t = scr.tile([128, CW], BF16, tag="om", name=f"om{s}")
            nc.vector.tensor_tensor(
                out=om_t, in0=o_t[:, C0:C0 + CW], in1=m_t, op=ALU.mult)
            for k in range(9):
                nc.tensor.matmul(
                    out=omsum_p, lhsT=vsel_s,
                    rhs=om_t[:, 512 * k:512 * (k + 1)],
                    start=(fl and k == 0), stop=(ll and k == 8))

        # drain persistent accumulators to SBUF, then DRAM
        nc.vector.tensor_copy(vs_sb[:, 0:512], osum_p)
        nc.vector.tensor_copy(vs_sb[:, 512:1024], eqsum_p)
        nc.vector.tensor_copy(vs_sb[:, 1024:1536], omsum_p)
        nc.default_dma_engine.dma_start(out=vs_d, in_=vs_sb)
        nc.default_dma_engine.dma_start(out=part_d[:, 0:6], in_=msum)
        nc.default_dma_engine.dma_start(out=part_d[:, 6:36], in_=edgesum)

    nc.compile()
    return nc


def _get_program():
    if "nc" not in _CACHE:
        _CACHE["nc"] = _build_program()
    return _CACHE["nc"]


def _make_in_maps(output, masks, loss_threshold):
    o5 = np.ascontiguousarray(np.asarray(output, dtype=np.float32)).reshape(
        NV, Z, H, W)
    m5 = np.ascontiguousarray(np.asarray(masks, dtype=np.float32)).reshape(
        NV, Z, H, W)
    thr = np.full((128, 1), np.float32(np.asarray(loss_threshold)), np.float32)
    cb = _consts()
    in_maps = []
    for c in range(NCORES):
        h0 = HC * c
        idx = np.clip(np.arange(h0 - 1, h0 + HC + 1), 0, H - 1)
        o_sh = np.ascontiguousarray(o5[:, :, idx, :]).reshape(NV * Z, FW)
        m_sh = np.ascontiguousarray(m5[:, :, h0:h0 + HC, :]).reshape(NV * Z, CW)
        in_maps.append({
            "o": o_sh, "m": m_sh, "thr": thr, "cb": cb,
        })
    return in_maps


def _combine(results):
    """Host-side tiny reduction: per-core partials -> loss scalar."""
    sum_m = np.zeros(NV)
    sum_eq = np.zeros(NV)
    sum_om = np.zeros(NV)
    sum_o = np.zeros(NV)
    sum_edge = np.zeros(NV)
    for r in results:
        p = np.asarray(r["partials"], dtype=np.float64)
        vs = np.asarray(r["vsums"], dtype=np.float64).reshape(NV, 3, 512)
        # [p, s]: volume = 2s + p//64, z = p%64
        sum_m += p[:, 0:6].reshape(2, 64, NS).sum(1).T.reshape(-1)
        sum_edge += (p[:, 6:36].reshape(2, 64, NS, 5).sum(axis=(1, 3))
                     .T.reshape(-1))
        sum_o += vs[:, 0].sum(-1)
        sum_eq += vs[:, 1].sum(-1)
        sum_om += vs[:, 2].sum(-1)

    freq = (sum_m / VOX).reshape(B, C)
    med = np.median(freq, axis=1, keepdims=True)
    w0 = 2.0 * med / (freq.min(axis=1, keepdims=True) + 1e-5)
    cw = (med / (freq + 1e-5)) * sum_eq.reshape(B, C) \
        + w0 * sum_edge.reshape(B, C)
    ps1 = sum_om.reshape(B, C)
    ps2 = (sum_o + sum_m).reshape(B, C)
    nom = (cw * ps1).sum(1)
    denom = (cw * ps2 + 1e-7).sum(1)
    loss = (1.0 - 2.0 * nom / denom).sum() / B
    return np.array([loss], dtype=np.float32)


def run(output, masks, loss_threshold, trace=False, **trace_kwargs):
    nc = _get_program()
    in_maps = _make_in_maps(output, masks, loss_threshold)
    res = run_bass_kernel_spmd(nc, in_maps, list(range(NCORES)),
                               trace=trace, **trace_kwargs)
    return _combine(res.results), res


def kernel(output, masks, loss_threshold):
    loss, _ = run(output, masks, loss_threshold)
    return loss
